# revision 22
# baseline (speedup 1.0000x reference)
"""DCNv4 Bass kernel for Trainium2, 8 NeuronCores, data-parallel over batch.

Per core (2 samples each), channels-on-partition / pixels-on-free layout:
  1. PE: value projection xv = value_w @ x  (into a zero-halo padded 60x60 grid)
  2. PE: grouped depthwise 3x3 conv as 9-tap paired matmuls (lhsT has the two
     per-channel weights at rows 2m/2m+1), col-tiled 2x64 so both output halves
     run concurrently; accumulated in PSUM f32.
  3. PE: offset/mask projection; om_w columns host-reordered to [ry | rx | mask]
     with grid offsets gy/gx and dw bias folded into the projection bias, so
     ry = gy + dy directly.  Features stored bf16.
  4. Tents t_k(u) = relu(1 - |u - k|), k in {-2,-1,1,2}; t_0 via partition of
     unity.  Computed PER PIXEL-CHUNK inside the sampling loop so they overlap
     with sampling of the previous chunk.  x-tents stored NEGATED
     (min(|u|-1, 0)) fully on DVE; the mask-fold uses -msk so the per-shift
     product (mty*txs) has the correct sign.
  5. PE: 0/1 matmul per shift = reduce-over-9-points + replicate-over-32-channels,
     giving composite weight A_s[(g,c), pix] in PSUM.
  6. DVE: A_s * xv_shifted products; PE: identity-matmul PSUM accumulation over
     the 25 composite shifts.  FSET shifts read A_s straight from PSUM (1x)
     skipping the ACT copy; GSET shifts compute their tent product on the
     otherwise-idle GPSIMD.  idm accumulation is emitted one shift late so the
     in-order PE never waits on the freshly produced vprod.
  7. PE: output projection, chunked with immediate DMA out.
"""

import numpy as np
import ml_dtypes

import concourse.bacc as bacc
import concourse.mybir as mybir
import concourse.tile as tile
from concourse.bass_utils import run_bass_kernel_spmd

F32 = mybir.dt.float32
BF16 = mybir.dt.bfloat16
AF = mybir.ActivationFunctionType
ALU = mybir.AluOpType

C, G, Cg = 256, 8, 32
N, H, W = 16, 56, 56
HW = H * W
NCORES = 8
S = N // NCORES
PAD = 2
Hp = Wp = H + 2 * PAD          # 60
KP = 72                        # G*9 point-features
NCH, NROWS = 448, 8            # projection N-chunk: 8 rows of 56
FCH, FROWS = 784, 14           # sampling-stage chunk: 14 rows of 56 (PSUM limit)
SHIFTS = [(sy, sx) for sy in range(-2, 3) for sx in range(-2, 3)]
FSET = {2, 12, 22}             # shifts whose vprod reads A from PSUM directly
GSET = {0, 5, 10, 15, 20}      # shifts whose tent product runs on GPSIMD


def _build_nc(dbg=False):
    nc = bacc.Bacc("TRN2", num_devices=NCORES)
    t_x = nc.dram_tensor("x", (S, C, H, W), F32, kind="ExternalInput")
    t_r = nc.dram_tensor("r", (S, C, H, W), F32, kind="ExternalInput")
    t_lv = nc.dram_tensor("lv", (C, C), BF16, kind="ExternalInput")
    t_lo = nc.dram_tensor("lo", (C, C), BF16, kind="ExternalInput")
    t_lom = nc.dram_tensor("lom", (C, 3 * KP), BF16, kind="ExternalInput")
    t_dwp = nc.dram_tensor("dwp", (36, 128, 64), BF16, kind="ExternalInput")
    t_bv = nc.dram_tensor("bv", (C, 1), F32, kind="ExternalInput")
    t_bo = nc.dram_tensor("bo", (C, 1), F32, kind="ExternalInput")
    t_bf = nc.dram_tensor("bf", (3 * KP, 1), F32, kind="ExternalInput")
    t_rep = nc.dram_tensor("rep", (2, KP, 128), BF16, kind="ExternalInput")
    t_kc = nc.dram_tensor("kc", (KP, 4), F32, kind="ExternalInput")
    t_id = nc.dram_tensor("idm", (128, 128), BF16, kind="ExternalInput")
    t_y = nc.dram_tensor("y", (S, C, H, W), F32, kind="ExternalOutput")
    dbg_t = {}
    if dbg:
        dbg_t["xv"] = nc.dram_tensor("dbg_xv", (S, C, Hp * Wp), F32, kind="ExternalOutput")
        dbg_t["dwf"] = nc.dram_tensor("dbg_dwf", (S, C, HW), F32, kind="ExternalOutput")
        dbg_t["feat"] = nc.dram_tensor("dbg_feat", (S, 3 * KP, HW), F32, kind="ExternalOutput")
        dbg_t["acc"] = nc.dram_tensor("dbg_acc", (S, C, HW), F32, kind="ExternalOutput")

    with tile.TileContext(nc) as tc:
        from contextlib import ExitStack
        ctx = ExitStack()
        wp = ctx.enter_context(tc.tile_pool(name="wts", bufs=1))
        lv = wp.tile([128, 2, C], BF16)
        nc.sync.dma_start(lv[:], t_lv[:].rearrange("(kc k) m -> k kc m", k=128))
        lo = wp.tile([128, 2, C], BF16)
        nc.sync.dma_start(lo[:], t_lo[:].rearrange("(kc k) m -> k kc m", k=128))
        lom = wp.tile([128, 2, 3 * KP], BF16)
        nc.sync.dma_start(lom[:], t_lom[:].rearrange("(kc k) m -> k kc m", k=128))
        dwp = wp.tile([128, 36, 64], BF16)
        nc.sync.dma_start(dwp[:], t_dwp[:].rearrange("n k m -> k n m"))
        bv = wp.tile([128, 2], F32)
        nc.sync.dma_start(bv[:], t_bv[:].rearrange("(mc k) o -> k (mc o)", k=128))
        bo = wp.tile([128, 2], F32)
        nc.sync.dma_start(bo[:], t_bo[:].rearrange("(mc k) o -> k (mc o)", k=128))
        bf = wp.tile([KP, 3], F32)
        nc.sync.dma_start(bf[:], t_bf[:].rearrange("(f k) o -> k (f o)", k=KP))
        rep = wp.tile([KP, 2, 128], BF16)
        nc.sync.dma_start(rep[:], t_rep[:].rearrange("g k m -> k g m"))
        idm = wp.tile([128, 128], BF16)
        nc.sync.dma_start(idm[:], t_id[:])
        kc = wp.tile([KP, 4], F32)
        nc.sync.dma_start(kc[:], t_kc[:])

        ap = ctx.enter_context(tc.tile_pool(name="acts", bufs=1))
        sp = ctx.enter_context(tc.tile_pool(name="small", bufs=3))
        sp4 = ctx.enter_context(tc.tile_pool(name="small4", bufs=4))
        pp = ctx.enter_context(tc.tile_pool(name="ps", bufs=2, space="PSUM"))
        pacc = ctx.enter_context(tc.tile_pool(name="pacc", bufs=1, space="PSUM"))

        def halo_zero(t):
            nc.vector.memset(t[:, 0:PAD, :], 0.0)
            nc.vector.memset(t[:, Hp - PAD : Hp, :], 0.0)
            nc.vector.memset(t[:, PAD : Hp - PAD, 0:PAD], 0.0)
            nc.vector.memset(t[:, PAD : Hp - PAD, Wp - PAD : Wp], 0.0)

        for s in range(S):
            x0 = ap.tile([128, Hp, Wp], BF16, tag="x0")
            x1 = ap.tile([128, Hp, Wp], BF16, tag="x1")
            r0 = ap.tile([128, Hp, Wp], BF16, tag="r0")
            r1 = ap.tile([128, Hp, Wp], BF16, tag="r1")
            for t in (x0, x1, r0, r1):
                nc.gpsimd.memset(t[:, 0:PAD, :], 0.0)
                nc.gpsimd.memset(t[:, Hp - PAD : Hp, :], 0.0)
                nc.gpsimd.memset(t[:, PAD : Hp - PAD, 0:PAD], 0.0)
                nc.gpsimd.memset(t[:, PAD : Hp - PAD, Wp - PAD : Wp], 0.0)
            src_x = t_x[s].rearrange("(par c) h w -> par c h w", par=2)
            src_r = t_r[s].rearrange("(par c) h w -> par c h w", par=2)
            nc.gpsimd.dma_start(x0[:, PAD : PAD + H, PAD : PAD + W], src_x[0])
            nc.gpsimd.dma_start(x1[:, PAD : PAD + H, PAD : PAD + W], src_x[1])
            nc.gpsimd.dma_start(r0[:, PAD : PAD + H, PAD : PAD + W], src_r[0])
            nc.gpsimd.dma_start(r1[:, PAD : PAD + H, PAD : PAD + W], src_r[1])

            def win(t, dy=0, dx=0, r0=0, nr=H):
                return t[:, PAD + dy + r0 : PAD + dy + r0 + nr, PAD + dx : PAD + dx + W]

            # ---- 1. value projection ----
            xvc = ap.tile([128, 2, Hp, Wp], BF16, tag="xv", name="xv")
            xv = [xvc[:, m] for m in range(2)]
            for m in range(2):
                halo_zero(xv[m])
            for m in range(2):
                for q in range(7):
                    ps = pp.tile([128, 1024], F32, tag="mm")
                    psv = ps[:, :NCH]
                    nc.tensor.matmul(psv, lv[:, 0, 128 * m : 128 * m + 128],
                                     win(x0, r0=NROWS * q, nr=NROWS), start=True, stop=False)
                    nc.tensor.matmul(psv, lv[:, 1, 128 * m : 128 * m + 128],
                                     win(x1, r0=NROWS * q, nr=NROWS), start=False, stop=True)
                    nc.scalar.activation(win(xv[m], r0=NROWS * q, nr=NROWS),
                                         psv.rearrange("p (h w) -> p h w", w=W),
                                         AF.Identity, bias=bv[:, m : m + 1], scale=1.0)

            # ---- 2. depthwise conv on PE: 9-tap paired matmuls, 2x64 col-tiled ----
            dwf = [ap.tile([128, HW], BF16, tag=f"dwf{b}", name=f"dwf{b}") for b in range(2)]
            for b, (ta, tb) in enumerate(((x0, x1), (r0, r1))):
                for q in range(7):
                    ps = pp.tile([128, 1024], F32, tag="mm")
                    psd = ps[:, :NCH]
                    for t in range(9):
                        i, j = t // 3, t % 3
                        for h, src in enumerate((ta, tb)):
                            nc.tensor.matmul(
                                psd[64 * h : 64 * h + 64, :],
                                dwp[:, b * 18 + h * 9 + t, :],
                                win(src, dy=i - 1, dx=j - 1, r0=NROWS * q, nr=NROWS),
                                start=(t == 0), stop=(t == 8),
                                tile_position=(0, 64 * h), skip_group_check=True)
                    nc.scalar.activation(dwf[b][:, NCH * q : NCH * q + NCH], psd,
                                         AF.Copy, scale=1.0)

            # xv shifted left by one column: makes every sampling-mul window
            # 4B-aligned (odd dx reads xvd at dx-1), keeping DVE in 2x mode
            xvdc = ap.tile([128, 2, Hp, Wp], BF16, tag="xvd", name="xvd")
            xvd = [xvdc[:, m] for m in range(2)]
            nc.vector.tensor_copy(xvdc[:, :, :, 0 : Wp - 1], xvc[:, :, :, 1:Wp])

            # ---- 3. offset/mask projection (features in bf16) ----
            feat = [ap.tile([KP, HW], BF16, tag=f"feat{f}", name=f"feat{f}") for f in range(3)]
            for f in range(3):
                for q in range(7):
                    ps = pp.tile([128, 1024], F32, tag="mm")
                    psf = ps[:KP, :NCH]
                    nc.tensor.matmul(psf, lom[:, 0, KP * f : KP * f + KP],
                                     dwf[0][:, NCH * q : NCH * q + NCH], start=True, stop=False)
                    nc.tensor.matmul(psf, lom[:, 1, KP * f : KP * f + KP],
                                     dwf[1][:, NCH * q : NCH * q + NCH], start=False, stop=True)
                    nc.scalar.activation(feat[f][:, NCH * q : NCH * q + NCH], psf,
                                         AF.Identity, bias=bf[:, f : f + 1], scale=1.0)
            ry, rx, msk = feat

            # ---- 4. tents (emitted per pixel-chunk inside the sampling loop) ----
            nmsk16 = ap.tile([KP, HW], BF16, tag="msk16")
            nc.vector.tensor_scalar(nmsk16[:], msk[:], -1.0, None, op0=ALU.mult)
            mty = {
                -2: ap.tile([KP, HW], BF16, tag="dwf0", name="mty-2"),
                -1: ap.tile([KP, HW], BF16, tag="dwf1", name="mty-1"),
                0: ap.tile([KP, HW], BF16, tag="mty0", name="mty0"),
                1: ap.tile([KP, HW], BF16, tag="mty1", name="mty1"),
                2: ap.tile([KP, HW], BF16, tag="mty2", name="mty2"),
            }
            txs = {
                -2: ap.tile([KP, HW], BF16, tag="r0", name="txs-2"),
                -1: ap.tile([KP, HW], BF16, tag="r1", name="txs-1"),
                0: ap.tile([KP, HW], BF16, tag="txs0", name="txs0"),
                1: ap.tile([KP, HW], BF16, tag="txs1", name="txs1"),
                2: ap.tile([KP, HW], BF16, tag="txs2", name="txs2"),
            }

            def emit_tents(col, ncol):
                rxc = rx[:, col : col + ncol]
                ryc = ry[:, col : col + ncol]
                # x-tents on DVE: -t_k = min(max(u-1, -u-1), 0), u = rx-k
                for k in (-2, -1, 1, 2):
                    th = sp.tile([KP, ncol], BF16, tag="tc16")
                    dst = txs[k][:, col : col + ncol]
                    nc.vector.tensor_scalar(th[:], rxc, float(-k - 1), None, op0=ALU.add)
                    nc.vector.tensor_scalar(dst, rxc, -1.0, float(k - 1), op0=ALU.mult, op1=ALU.add)
                    nc.vector.tensor_tensor(dst, dst, th[:], op=ALU.max)
                    nc.vector.tensor_scalar(dst, dst, 0.0, None, op0=ALU.min)
                # y-tents on ACT, negated-mask fold on DVE
                for kj, k in enumerate((-2, -1, 1, 2)):
                    kb = kc[:, kj : kj + 1]
                    tmpc = sp.tile([KP, ncol], BF16, tag="tmpc")
                    t16 = sp.tile([KP, ncol], BF16, tag="tc16")
                    nc.scalar.activation(tmpc[:], ryc, AF.Abs, bias=kb, scale=1.0)
                    nc.scalar.activation(t16[:], tmpc[:], AF.Relu, bias=1.0, scale=-1.0)
                    nc.vector.tensor_mul(mty[k][:, col : col + ncol],
                                         nmsk16[:, col : col + ncol], t16[:])
                # partition of unity
                s01 = sp.tile([KP, ncol], BF16, tag="s01c")
                nc.vector.tensor_add(s01[:], mty[-2][:, col : col + ncol], mty[-1][:, col : col + ncol])
                nc.vector.tensor_add(s01[:], s01[:], mty[1][:, col : col + ncol])
                nc.vector.tensor_add(s01[:], s01[:], mty[2][:, col : col + ncol])
                nc.vector.tensor_tensor(mty[0][:, col : col + ncol],
                                        nmsk16[:, col : col + ncol], s01[:], op=ALU.subtract)
                s01b = sp.tile([KP, ncol], BF16, tag="s01c")
                nc.vector.tensor_add(s01b[:], txs[-2][:, col : col + ncol], txs[-1][:, col : col + ncol])
                nc.vector.tensor_add(s01b[:], s01b[:], txs[1][:, col : col + ncol])
                nc.vector.tensor_add(s01b[:], s01b[:], txs[2][:, col : col + ncol])
                nc.scalar.activation(txs[0][:, col : col + ncol], s01b[:],
                                     AF.Copy, bias=-1.0, scale=-1.0)

            # ---- 5+6. deformable sampling ----
            acc16 = [ap.tile([128, HW], BF16, tag=f"acc16_{gb}", name=f"acc16_{gb}") for gb in range(2)]
            for hc in range(4):
                col = FCH * hc
                emit_tents(col, FCH)
                accp = [[pacc.tile([128, 392], F32, tag=f"acc{gb}{hh}", name=f"accp{gb}{hh}")
                         for hh in range(2)] for gb in range(2)]

                def emit_prod(si):
                    sy, sx = SHIFTS[si]
                    prod = sp.tile([KP, FCH], BF16, tag="prod")
                    src0 = mty[sy][:, col : col + FCH]
                    src1 = txs[sx][:, col : col + FCH]
                    if si in GSET:
                        nc.gpsimd.tensor_tensor(prod[:], src0, src1, op=ALU.mult)
                    else:
                        nc.vector.tensor_mul(prod[:], src0, src1)
                    return prod

                def emit_idm(psi, pv, last=False):
                    for gb in range(2):
                        for hh in range(2):
                            nc.tensor.matmul(accp[gb][hh][:], idm[:],
                                             pv[:, gb, 392 * hh : 392 * hh + 392],
                                             start=(psi == 0), stop=last)

                pend = []   # [(si, vpcat)] awaiting idm accumulation (depth 2)
                prods = {0: emit_prod(0)}
                for si, (sy, sx) in enumerate(SHIFTS):
                    if si + 1 < len(SHIFTS):
                        prods[si + 1] = emit_prod(si + 1)
                    prod = prods.pop(si)
                    vpcat = sp4.tile([128, 2, FCH], BF16, tag="vp")
                    # windows over BOTH gb halves at once: [p, 2, FROWS, W]
                    r0_ = PAD + sy + FROWS * hc
                    c0_ = PAD + sx if sx % 2 == 0 else PAD + sx - 1
                    xsv = xvc if sx % 2 == 0 else xvdc
                    xsrc2 = xsv[:, :, r0_ : r0_ + FROWS, c0_ : c0_ + W]
                    apss = []
                    for gb in range(2):
                        aps = pp.tile([128, 1024], F32, tag="mm")
                        for hh in range(2):
                            nc.tensor.matmul(aps[:, 512 * hh : 512 * hh + 392], rep[:, gb, :],
                                             prod[:, 392 * hh : 392 * hh + 392],
                                             start=True, stop=True)
                        apss.append(aps)
                    if si in FSET:
                        # direct-PSUM read on DVE (1x), skipping the ACT copy
                        for gb in range(2):
                            av = apss[gb][:].rearrange("p (b f) -> p b f", f=512)[:, :, :392] \
                                .rearrange("p b (r w) -> p b r w", w=W)
                            nc.vector.tensor_tensor(
                                vpcat[:, gb].rearrange("p (b r w) -> p b r w", r=FROWS // 2, w=W),
                                av,
                                xsrc2[:, gb].rearrange("p (b r) w -> p b r w", b=2),
                                op=ALU.mult)
                    else:
                        a16c = sp.tile([128, 2, FCH], BF16, tag="a16")
                        for gb in range(2):
                            nc.scalar.activation(
                                a16c[:, gb].rearrange("p (b f) -> p b f", f=392),
                                apss[gb][:].rearrange("p (b f) -> p b f", f=512)[:, :, :392],
                                AF.Copy, scale=1.0)
                        # ONE 2x-mode multiply over both gb halves (FD=1568)
                        nc.vector.tensor_tensor(
                            vpcat[:].rearrange("p g (r w) -> p g r w", w=W),
                            a16c[:].rearrange("p g (r w) -> p g r w", w=W),
                            xsrc2, op=ALU.mult)
                    pend.append((si, vpcat))
                    # accumulate two shifts late so the in-order PE never waits
                    if len(pend) > 2:
                        emit_idm(*pend.pop(0))
                for psi, pvps in pend:
                    emit_idm(psi, pvps, last=(psi == len(SHIFTS) - 1))
                for gb in range(2):
                    for hh in range(2):
                        nc.scalar.activation(acc16[gb][:, col + 392 * hh : col + 392 * hh + 392],
                                             accp[gb][hh][:], AF.Copy, scale=1.0)

            # ---- 7. output projection, chunked with immediate DMA ----
            yq = t_y[s].rearrange("(mc k) h w -> mc k h w", mc=2)
            for m in range(2):
                for q in range(7):
                    ps = pp.tile([128, 1024], F32, tag="mm")
                    psv = ps[:, :NCH]
                    nc.tensor.matmul(psv, lo[:, 0, 128 * m : 128 * m + 128],
                                     acc16[0][:, NCH * q : NCH * q + NCH], start=True, stop=False)
                    nc.tensor.matmul(psv, lo[:, 1, 128 * m : 128 * m + 128],
                                     acc16[1][:, NCH * q : NCH * q + NCH], start=False, stop=True)
                    ytc = sp.tile([128, NCH], F32, tag="ytc")
                    nc.scalar.activation(ytc[:], psv, AF.Identity, bias=bo[:, m : m + 1], scale=1.0)
                    nc.sync.dma_start(yq[m, :, NROWS * q : NROWS * q + NROWS, :],
                                      ytc[:].rearrange("p (h w) -> p h w", w=W))

            if dbg:
                for m in range(2):
                    nc.gpsimd.dma_start(dbg_t["xv"][s, 128 * m : 128 * m + 128],
                                      xv[m][:].rearrange("p h w -> p (h w)"))
                    nc.gpsimd.dma_start(dbg_t["dwf"][s, 128 * m : 128 * m + 128], dwf[m][:])
                    nc.gpsimd.dma_start(dbg_t["acc"][s, 128 * m : 128 * m + 128], acc16[m][:])
                for f in range(3):
                    nc.sync.dma_start(dbg_t["feat"][s, KP * f : KP * f + KP], feat[f][:])
        ctx.close()
    nc.compile()
    return nc


def _prep_weights(inputs):
    value_w = np.asarray(inputs["value_w"], np.float32)
    out_w = np.asarray(inputs["out_w"], np.float32)
    om_w = np.asarray(inputs["om_w"], np.float32)
    dw_w = np.asarray(inputs["dw_w"], np.float32)
    value_b = np.asarray(inputs["value_b"], np.float32)
    out_b = np.asarray(inputs["out_b"], np.float32)
    om_b = np.asarray(inputs["om_b"], np.float32)
    dw_b = np.asarray(inputs["dw_b"], np.float32)

    lv = value_w.T
    lo = out_w.T

    gidx = np.arange(G)[:, None]
    p = np.arange(9)[None, :]
    col_ry = (gidx * 27 + 2 * p + 1).reshape(-1)
    col_rx = (gidx * 27 + 2 * p).reshape(-1)
    col_mk = (gidx * 27 + 18 + p).reshape(-1)
    cols = np.concatenate([col_ry, col_rx, col_mk])
    lom = om_w.T[:, cols]
    gy = np.tile((np.arange(9) // 3 - 1).astype(np.float32), G)
    gx = np.tile((np.arange(9) % 3 - 1).astype(np.float32), G)
    bfeat = np.concatenate([om_b[col_ry] + gy, om_b[col_rx] + gx, om_b[col_mk]])
    bfeat = bfeat + (om_w @ dw_b)[cols]

    # paired depthwise weights: dwp[b*18 + h*9 + t] is [128 src-part, 64 out]
    # dw output channel cbase+m uses src channels 2m, 2m+1 of its half-tile
    dww = dw_w.reshape(C, 2, 9)
    dwp = np.zeros((36, 128, 64), np.float32)
    for b in range(2):          # dwf0 <- input halves, dwf1 <- ref halves
        for h in range(2):      # out rows 64h..64h+63, src tile (x|r)[h]
            cbase = b * 128 + h * 64
            for t in range(9):
                mat = dwp[b * 18 + h * 9 + t]
                m = np.arange(64)
                mat[2 * m, m] = dww[cbase + m, 0, t]
                mat[2 * m + 1, m] = dww[cbase + m, 1, t]

    rep = np.zeros((2, KP, 128), np.float32)
    for gb in range(2):
        for k in range(KP):
            g = k // 9
            if gb * 4 <= g < gb * 4 + 4:
                m0 = (g - gb * 4) * 32
                rep[gb, k, m0 : m0 + 32] = 1.0

    b16 = lambda a: np.ascontiguousarray(np.asarray(a, np.float32)).astype(ml_dtypes.bfloat16)
    f32 = lambda a: np.ascontiguousarray(np.asarray(a, np.float32))
    return {
        "lv": b16(lv), "lo": b16(lo), "lom": b16(lom),
        "dwp": b16(dwp),
        "bv": f32(value_b).reshape(C, 1), "bo": f32(out_b).reshape(C, 1),
        "bf": f32(bfeat).reshape(3 * KP, 1),
        "rep": b16(rep), "idm": b16(np.eye(128, dtype=np.float32)),
        "kc": np.tile(np.array([2.0, 1.0, -1.0, -2.0], np.float32), (KP, 1)),
    }


_CACHE = {}


def kernel(**inputs):
    dbg = bool(inputs.pop("_dbg", False))
    trace = bool(inputs.pop("_trace", False))
    x = np.ascontiguousarray(np.asarray(inputs["input"], np.float32))
    r = np.ascontiguousarray(np.asarray(inputs["ref"], np.float32))
    wts = _prep_weights(inputs)

    key = ("nc", dbg)
    if key not in _CACHE:
        _CACHE[key] = _build_nc(dbg=dbg)
    nc = _CACHE[key]

    in_maps = []
    for c in range(NCORES):
        m = dict(wts)
        m["x"] = np.ascontiguousarray(x[c * S : (c + 1) * S])
        m["r"] = np.ascontiguousarray(r[c * S : (c + 1) * S])
        in_maps.append(m)

    res = run_bass_kernel_spmd(nc, in_maps, core_ids=list(range(NCORES)), trace=trace)
    kernel.last_results = res
    kernel.last_exec_ns = res.exec_time_ns
    y = np.concatenate([res.results[c]["y"] for c in range(NCORES)], axis=0)
    return y.reshape(N, C, H, W)


# revision 23
# speedup vs baseline: 1.0070x; 1.0070x over previous
"""DCNv4 Bass kernel for Trainium2, 8 NeuronCores, data-parallel over batch.

Per core (2 samples each), channels-on-partition / pixels-on-free layout:
  1. PE: value projection xv = value_w @ x  (into a zero-halo padded 60x60 grid)
  2. PE: grouped depthwise 3x3 conv as 9-tap paired matmuls (lhsT has the two
     per-channel weights at rows 2m/2m+1), col-tiled 2x64 so both output halves
     run concurrently; accumulated in PSUM f32.
  3. PE: offset/mask projection; om_w columns host-reordered to [ry | rx | mask]
     with grid offsets gy/gx and dw bias folded into the projection bias, so
     ry = gy + dy directly.  Features stored bf16.
  4. Tents t_k(u) = relu(1 - |u - k|), k in {-2,-1,1,2}; t_0 via partition of
     unity.  Computed PER PIXEL-CHUNK inside the sampling loop so they overlap
     with sampling of the previous chunk.  x-tents stored NEGATED
     (min(|u|-1, 0)) fully on DVE; the mask-fold uses -msk so the per-shift
     product (mty*txs) has the correct sign.
  5. PE: 0/1 matmul per shift = reduce-over-9-points + replicate-over-32-channels,
     giving composite weight A_s[(g,c), pix] in PSUM.
  6. DVE: A_s * xv_shifted products; PE: identity-matmul PSUM accumulation over
     the 25 composite shifts.  FSET shifts read A_s straight from PSUM (1x)
     skipping the ACT copy; GSET shifts compute their tent product on the
     otherwise-idle GPSIMD.  idm accumulation is emitted one shift late so the
     in-order PE never waits on the freshly produced vprod.
  7. PE: output projection, chunked with immediate DMA out.
"""

import numpy as np
import ml_dtypes

import concourse.bacc as bacc
import concourse.mybir as mybir
import concourse.tile as tile
from concourse.bass_utils import run_bass_kernel_spmd

F32 = mybir.dt.float32
BF16 = mybir.dt.bfloat16
AF = mybir.ActivationFunctionType
ALU = mybir.AluOpType

C, G, Cg = 256, 8, 32
N, H, W = 16, 56, 56
HW = H * W
NCORES = 8
S = N // NCORES
PAD = 2
Hp = Wp = H + 2 * PAD          # 60
KP = 72                        # G*9 point-features
NCH, NROWS = 448, 8            # projection N-chunk: 8 rows of 56
FCH, FROWS = 784, 14           # sampling-stage chunk: 14 rows of 56 (PSUM limit)
SHIFTS = [(sy, sx) for sy in range(-2, 3) for sx in range(-2, 3)]
FSET = {2, 12, 22}             # shifts whose vprod reads A from PSUM directly
GSET = {0, 5, 10, 15, 20}      # shifts whose tent product runs on GPSIMD


def _build_nc(dbg=False):
    nc = bacc.Bacc("TRN2", num_devices=NCORES)
    t_x = nc.dram_tensor("x", (S, C, H, W), F32, kind="ExternalInput")
    t_r = nc.dram_tensor("r", (S, C, H, W), F32, kind="ExternalInput")
    t_lv = nc.dram_tensor("lv", (C, C), BF16, kind="ExternalInput")
    t_lo = nc.dram_tensor("lo", (C, C), BF16, kind="ExternalInput")
    t_lom = nc.dram_tensor("lom", (C, 3 * KP), BF16, kind="ExternalInput")
    t_dwp = nc.dram_tensor("dwp", (36, 128, 64), BF16, kind="ExternalInput")
    t_bv = nc.dram_tensor("bv", (C, 1), F32, kind="ExternalInput")
    t_bo = nc.dram_tensor("bo", (C, 1), F32, kind="ExternalInput")
    t_bf = nc.dram_tensor("bf", (3 * KP, 1), F32, kind="ExternalInput")
    t_rep = nc.dram_tensor("rep", (2, KP, 128), BF16, kind="ExternalInput")
    t_kc = nc.dram_tensor("kc", (KP, 4), F32, kind="ExternalInput")
    t_id = nc.dram_tensor("idm", (128, 128), BF16, kind="ExternalInput")
    t_y = nc.dram_tensor("y", (S, C, H, W), F32, kind="ExternalOutput")
    dbg_t = {}
    if dbg:
        dbg_t["xv"] = nc.dram_tensor("dbg_xv", (S, C, Hp * Wp), F32, kind="ExternalOutput")
        dbg_t["dwf"] = nc.dram_tensor("dbg_dwf", (S, C, HW), F32, kind="ExternalOutput")
        dbg_t["feat"] = nc.dram_tensor("dbg_feat", (S, 3 * KP, HW), F32, kind="ExternalOutput")
        dbg_t["acc"] = nc.dram_tensor("dbg_acc", (S, C, HW), F32, kind="ExternalOutput")

    with tile.TileContext(nc) as tc:
        from contextlib import ExitStack
        ctx = ExitStack()
        wp = ctx.enter_context(tc.tile_pool(name="wts", bufs=1))
        lv = wp.tile([128, 2, C], BF16)
        nc.sync.dma_start(lv[:], t_lv[:].rearrange("(kc k) m -> k kc m", k=128))
        lo = wp.tile([128, 2, C], BF16)
        nc.sync.dma_start(lo[:], t_lo[:].rearrange("(kc k) m -> k kc m", k=128))
        lom = wp.tile([128, 2, 3 * KP], BF16)
        nc.sync.dma_start(lom[:], t_lom[:].rearrange("(kc k) m -> k kc m", k=128))
        dwp = wp.tile([128, 36, 64], BF16)
        nc.sync.dma_start(dwp[:], t_dwp[:].rearrange("n k m -> k n m"))
        bv = wp.tile([128, 2], F32)
        nc.sync.dma_start(bv[:], t_bv[:].rearrange("(mc k) o -> k (mc o)", k=128))
        bo = wp.tile([128, 2], F32)
        nc.sync.dma_start(bo[:], t_bo[:].rearrange("(mc k) o -> k (mc o)", k=128))
        bf = wp.tile([KP, 3], F32)
        nc.sync.dma_start(bf[:], t_bf[:].rearrange("(f k) o -> k (f o)", k=KP))
        rep = wp.tile([KP, 2, 128], BF16)
        nc.sync.dma_start(rep[:], t_rep[:].rearrange("g k m -> k g m"))
        idm = wp.tile([128, 128], BF16)
        nc.sync.dma_start(idm[:], t_id[:])
        kc = wp.tile([KP, 4], F32)
        nc.sync.dma_start(kc[:], t_kc[:])

        ap = ctx.enter_context(tc.tile_pool(name="acts", bufs=1))
        sp = ctx.enter_context(tc.tile_pool(name="small", bufs=3))
        sp4 = ctx.enter_context(tc.tile_pool(name="small4", bufs=4))
        pp = ctx.enter_context(tc.tile_pool(name="ps", bufs=2, space="PSUM"))
        pacc = ctx.enter_context(tc.tile_pool(name="pacc", bufs=1, space="PSUM"))

        def halo_zero(t):
            nc.vector.memset(t[:, 0:PAD, :], 0.0)
            nc.vector.memset(t[:, Hp - PAD : Hp, :], 0.0)
            nc.vector.memset(t[:, PAD : Hp - PAD, 0:PAD], 0.0)
            nc.vector.memset(t[:, PAD : Hp - PAD, Wp - PAD : Wp], 0.0)

        for s in range(S):
            x0 = ap.tile([128, Hp, Wp], BF16, tag="x0")
            x1 = ap.tile([128, Hp, Wp], BF16, tag="x1")
            r0 = ap.tile([128, Hp, Wp], BF16, tag="r0")
            r1 = ap.tile([128, Hp, Wp], BF16, tag="r1")
            for t in (x0, x1, r0, r1):
                halo_zero(t)
            src_x = t_x[s].rearrange("(par c) h w -> par c h w", par=2)
            src_r = t_r[s].rearrange("(par c) h w -> par c h w", par=2)
            nc.gpsimd.dma_start(x0[:, PAD : PAD + H, PAD : PAD + W], src_x[0])
            nc.gpsimd.dma_start(x1[:, PAD : PAD + H, PAD : PAD + W], src_x[1])
            nc.gpsimd.dma_start(r0[:, PAD : PAD + H, PAD : PAD + W], src_r[0])
            nc.gpsimd.dma_start(r1[:, PAD : PAD + H, PAD : PAD + W], src_r[1])

            def win(t, dy=0, dx=0, r0=0, nr=H):
                return t[:, PAD + dy + r0 : PAD + dy + r0 + nr, PAD + dx : PAD + dx + W]

            # ---- 1. value projection ----
            xvc = ap.tile([128, 2, Hp, Wp], BF16, tag="xv", name="xv")
            xv = [xvc[:, m] for m in range(2)]
            for m in range(2):
                halo_zero(xv[m])
            for m in range(2):
                for q in range(7):
                    ps = pp.tile([128, 1024], F32, tag="mm")
                    psv = ps[:, :NCH]
                    nc.tensor.matmul(psv, lv[:, 0, 128 * m : 128 * m + 128],
                                     win(x0, r0=NROWS * q, nr=NROWS), start=True, stop=False)
                    nc.tensor.matmul(psv, lv[:, 1, 128 * m : 128 * m + 128],
                                     win(x1, r0=NROWS * q, nr=NROWS), start=False, stop=True)
                    nc.scalar.activation(win(xv[m], r0=NROWS * q, nr=NROWS),
                                         psv.rearrange("p (h w) -> p h w", w=W),
                                         AF.Identity, bias=bv[:, m : m + 1], scale=1.0)

            # ---- 2. depthwise conv on PE: 9-tap paired matmuls, 2x64 col-tiled ----
            dwf = [ap.tile([128, HW], BF16, tag=f"dwf{b}", name=f"dwf{b}") for b in range(2)]
            for b, (ta, tb) in enumerate(((x0, x1), (r0, r1))):
                for q in range(7):
                    ps = pp.tile([128, 1024], F32, tag="mm")
                    psd = ps[:, :NCH]
                    for t in range(9):
                        i, j = t // 3, t % 3
                        for h, src in enumerate((ta, tb)):
                            nc.tensor.matmul(
                                psd[64 * h : 64 * h + 64, :],
                                dwp[:, b * 18 + h * 9 + t, :],
                                win(src, dy=i - 1, dx=j - 1, r0=NROWS * q, nr=NROWS),
                                start=(t == 0), stop=(t == 8),
                                tile_position=(0, 64 * h), skip_group_check=True)
                    nc.scalar.activation(dwf[b][:, NCH * q : NCH * q + NCH], psd,
                                         AF.Copy, scale=1.0)

            # xv shifted left by one column: makes every sampling-mul window
            # 4B-aligned (odd dx reads xvd at dx-1), keeping DVE in 2x mode
            xvdc = ap.tile([128, 2, Hp, Wp], BF16, tag="xvd", name="xvd")
            xvd = [xvdc[:, m] for m in range(2)]
            nc.vector.tensor_copy(xvdc[:, :, :, 0 : Wp - 1], xvc[:, :, :, 1:Wp])

            # ---- 3. offset/mask projection (features in bf16) ----
            feat = [ap.tile([KP, HW], BF16, tag=f"feat{f}", name=f"feat{f}") for f in range(3)]
            for f in range(3):
                for q in range(7):
                    ps = pp.tile([128, 1024], F32, tag="mm")
                    psf = ps[:KP, :NCH]
                    nc.tensor.matmul(psf, lom[:, 0, KP * f : KP * f + KP],
                                     dwf[0][:, NCH * q : NCH * q + NCH], start=True, stop=False)
                    nc.tensor.matmul(psf, lom[:, 1, KP * f : KP * f + KP],
                                     dwf[1][:, NCH * q : NCH * q + NCH], start=False, stop=True)
                    nc.scalar.activation(feat[f][:, NCH * q : NCH * q + NCH], psf,
                                         AF.Identity, bias=bf[:, f : f + 1], scale=1.0)
            ry, rx, msk = feat

            # ---- 4. tents (emitted per pixel-chunk inside the sampling loop) ----
            nmsk16 = ap.tile([KP, HW], BF16, tag="msk16")
            nc.vector.tensor_scalar(nmsk16[:], msk[:], -1.0, None, op0=ALU.mult)
            mty = {
                -2: ap.tile([KP, HW], BF16, tag="dwf0", name="mty-2"),
                -1: ap.tile([KP, HW], BF16, tag="dwf1", name="mty-1"),
                0: ap.tile([KP, HW], BF16, tag="mty0", name="mty0"),
                1: ap.tile([KP, HW], BF16, tag="mty1", name="mty1"),
                2: ap.tile([KP, HW], BF16, tag="mty2", name="mty2"),
            }
            txs = {
                -2: ap.tile([KP, HW], BF16, tag="r0", name="txs-2"),
                -1: ap.tile([KP, HW], BF16, tag="r1", name="txs-1"),
                0: ap.tile([KP, HW], BF16, tag="txs0", name="txs0"),
                1: ap.tile([KP, HW], BF16, tag="txs1", name="txs1"),
                2: ap.tile([KP, HW], BF16, tag="txs2", name="txs2"),
            }

            def emit_tents(col, ncol):
                rxc = rx[:, col : col + ncol]
                ryc = ry[:, col : col + ncol]
                # x-tents on DVE: -t_k = min(max(u-1, -u-1), 0), u = rx-k
                for k in (-2, -1, 1, 2):
                    th = sp.tile([KP, ncol], BF16, tag="tc16")
                    dst = txs[k][:, col : col + ncol]
                    nc.vector.tensor_scalar(th[:], rxc, float(-k - 1), None, op0=ALU.add)
                    nc.vector.tensor_scalar(dst, rxc, -1.0, float(k - 1), op0=ALU.mult, op1=ALU.add)
                    nc.vector.tensor_tensor(dst, dst, th[:], op=ALU.max)
                    nc.vector.tensor_scalar(dst, dst, 0.0, None, op0=ALU.min)
                # y-tents on ACT, negated-mask fold on DVE
                for kj, k in enumerate((-2, -1, 1, 2)):
                    kb = kc[:, kj : kj + 1]
                    tmpc = sp.tile([KP, ncol], BF16, tag="tmpc")
                    t16 = sp.tile([KP, ncol], BF16, tag="tc16")
                    nc.scalar.activation(tmpc[:], ryc, AF.Abs, bias=kb, scale=1.0)
                    nc.scalar.activation(t16[:], tmpc[:], AF.Relu, bias=1.0, scale=-1.0)
                    nc.vector.tensor_mul(mty[k][:, col : col + ncol],
                                         nmsk16[:, col : col + ncol], t16[:])
                # partition of unity
                s01 = sp.tile([KP, ncol], BF16, tag="s01c")
                nc.vector.tensor_add(s01[:], mty[-2][:, col : col + ncol], mty[-1][:, col : col + ncol])
                nc.vector.tensor_add(s01[:], s01[:], mty[1][:, col : col + ncol])
                nc.vector.tensor_add(s01[:], s01[:], mty[2][:, col : col + ncol])
                nc.vector.tensor_tensor(mty[0][:, col : col + ncol],
                                        nmsk16[:, col : col + ncol], s01[:], op=ALU.subtract)
                s01b = sp.tile([KP, ncol], BF16, tag="s01c")
                nc.vector.tensor_add(s01b[:], txs[-2][:, col : col + ncol], txs[-1][:, col : col + ncol])
                nc.vector.tensor_add(s01b[:], s01b[:], txs[1][:, col : col + ncol])
                nc.vector.tensor_add(s01b[:], s01b[:], txs[2][:, col : col + ncol])
                nc.scalar.activation(txs[0][:, col : col + ncol], s01b[:],
                                     AF.Copy, bias=-1.0, scale=-1.0)

            # ---- 5+6. deformable sampling ----
            acc16 = [ap.tile([128, HW], BF16, tag=f"acc16_{gb}", name=f"acc16_{gb}") for gb in range(2)]
            for hc in range(4):
                col = FCH * hc
                emit_tents(col, FCH)
                accp = [[pacc.tile([128, 392], F32, tag=f"acc{gb}{hh}", name=f"accp{gb}{hh}")
                         for hh in range(2)] for gb in range(2)]

                def emit_prod(si):
                    sy, sx = SHIFTS[si]
                    prod = sp.tile([KP, FCH], BF16, tag="prod")
                    src0 = mty[sy][:, col : col + FCH]
                    src1 = txs[sx][:, col : col + FCH]
                    if si in GSET:
                        nc.gpsimd.tensor_tensor(prod[:], src0, src1, op=ALU.mult)
                    else:
                        nc.vector.tensor_mul(prod[:], src0, src1)
                    return prod

                def emit_idm(psi, pv, last=False):
                    for gb in range(2):
                        for hh in range(2):
                            nc.tensor.matmul(accp[gb][hh][:], idm[:],
                                             pv[:, gb, 392 * hh : 392 * hh + 392],
                                             start=(psi == 0), stop=last)

                pend = []   # [(si, vpcat)] awaiting idm accumulation (depth 2)
                prods = {0: emit_prod(0)}
                for si, (sy, sx) in enumerate(SHIFTS):
                    if si + 1 < len(SHIFTS):
                        prods[si + 1] = emit_prod(si + 1)
                    prod = prods.pop(si)
                    vpcat = sp4.tile([128, 2, FCH], BF16, tag="vp")
                    # windows over BOTH gb halves at once: [p, 2, FROWS, W]
                    r0_ = PAD + sy + FROWS * hc
                    c0_ = PAD + sx if sx % 2 == 0 else PAD + sx - 1
                    xsv = xvc if sx % 2 == 0 else xvdc
                    xsrc2 = xsv[:, :, r0_ : r0_ + FROWS, c0_ : c0_ + W]
                    apss = []
                    for gb in range(2):
                        aps = pp.tile([128, 1024], F32, tag="mm")
                        for hh in range(2):
                            nc.tensor.matmul(aps[:, 512 * hh : 512 * hh + 392], rep[:, gb, :],
                                             prod[:, 392 * hh : 392 * hh + 392],
                                             start=True, stop=True)
                        apss.append(aps)
                    if si in FSET:
                        # direct-PSUM read on DVE (1x), skipping the ACT copy
                        for gb in range(2):
                            av = apss[gb][:].rearrange("p (b f) -> p b f", f=512)[:, :, :392] \
                                .rearrange("p b (r w) -> p b r w", w=W)
                            nc.vector.tensor_tensor(
                                vpcat[:, gb].rearrange("p (b r w) -> p b r w", r=FROWS // 2, w=W),
                                av,
                                xsrc2[:, gb].rearrange("p (b r) w -> p b r w", b=2),
                                op=ALU.mult)
                    else:
                        a16c = sp.tile([128, 2, FCH], BF16, tag="a16")
                        for gb in range(2):
                            nc.scalar.activation(
                                a16c[:, gb].rearrange("p (b f) -> p b f", f=392),
                                apss[gb][:].rearrange("p (b f) -> p b f", f=512)[:, :, :392],
                                AF.Copy, scale=1.0)
                        # ONE 2x-mode multiply over both gb halves (FD=1568)
                        nc.vector.tensor_tensor(
                            vpcat[:].rearrange("p g (r w) -> p g r w", w=W),
                            a16c[:].rearrange("p g (r w) -> p g r w", w=W),
                            xsrc2, op=ALU.mult)
                    pend.append((si, vpcat))
                    # accumulate two shifts late so the in-order PE never waits
                    if len(pend) > 2:
                        emit_idm(*pend.pop(0))
                for psi, pvps in pend:
                    emit_idm(psi, pvps, last=(psi == len(SHIFTS) - 1))
                for gb in range(2):
                    for hh in range(2):
                        nc.scalar.activation(acc16[gb][:, col + 392 * hh : col + 392 * hh + 392],
                                             accp[gb][hh][:], AF.Copy, scale=1.0)

            # ---- 7. output projection, chunked with immediate DMA ----
            yq = t_y[s].rearrange("(mc k) h w -> mc k h w", mc=2)
            for m in range(2):
                for q in range(7):
                    ps = pp.tile([128, 1024], F32, tag="mm")
                    psv = ps[:, :NCH]
                    nc.tensor.matmul(psv, lo[:, 0, 128 * m : 128 * m + 128],
                                     acc16[0][:, NCH * q : NCH * q + NCH], start=True, stop=False)
                    nc.tensor.matmul(psv, lo[:, 1, 128 * m : 128 * m + 128],
                                     acc16[1][:, NCH * q : NCH * q + NCH], start=False, stop=True)
                    ytc = sp.tile([128, NCH], F32, tag="ytc")
                    nc.scalar.activation(ytc[:], psv, AF.Identity, bias=bo[:, m : m + 1], scale=1.0)
                    nc.sync.dma_start(yq[m, :, NROWS * q : NROWS * q + NROWS, :],
                                      ytc[:].rearrange("p (h w) -> p h w", w=W))

            if dbg:
                for m in range(2):
                    nc.gpsimd.dma_start(dbg_t["xv"][s, 128 * m : 128 * m + 128],
                                      xv[m][:].rearrange("p h w -> p (h w)"))
                    nc.gpsimd.dma_start(dbg_t["dwf"][s, 128 * m : 128 * m + 128], dwf[m][:])
                    nc.gpsimd.dma_start(dbg_t["acc"][s, 128 * m : 128 * m + 128], acc16[m][:])
                for f in range(3):
                    nc.sync.dma_start(dbg_t["feat"][s, KP * f : KP * f + KP], feat[f][:])
        ctx.close()
    nc.compile()
    return nc


def _prep_weights(inputs):
    value_w = np.asarray(inputs["value_w"], np.float32)
    out_w = np.asarray(inputs["out_w"], np.float32)
    om_w = np.asarray(inputs["om_w"], np.float32)
    dw_w = np.asarray(inputs["dw_w"], np.float32)
    value_b = np.asarray(inputs["value_b"], np.float32)
    out_b = np.asarray(inputs["out_b"], np.float32)
    om_b = np.asarray(inputs["om_b"], np.float32)
    dw_b = np.asarray(inputs["dw_b"], np.float32)

    lv = value_w.T
    lo = out_w.T

    gidx = np.arange(G)[:, None]
    p = np.arange(9)[None, :]
    col_ry = (gidx * 27 + 2 * p + 1).reshape(-1)
    col_rx = (gidx * 27 + 2 * p).reshape(-1)
    col_mk = (gidx * 27 + 18 + p).reshape(-1)
    cols = np.concatenate([col_ry, col_rx, col_mk])
    lom = om_w.T[:, cols]
    gy = np.tile((np.arange(9) // 3 - 1).astype(np.float32), G)
    gx = np.tile((np.arange(9) % 3 - 1).astype(np.float32), G)
    bfeat = np.concatenate([om_b[col_ry] + gy, om_b[col_rx] + gx, om_b[col_mk]])
    bfeat = bfeat + (om_w @ dw_b)[cols]

    # paired depthwise weights: dwp[b*18 + h*9 + t] is [128 src-part, 64 out]
    # dw output channel cbase+m uses src channels 2m, 2m+1 of its half-tile
    dww = dw_w.reshape(C, 2, 9)
    dwp = np.zeros((36, 128, 64), np.float32)
    for b in range(2):          # dwf0 <- input halves, dwf1 <- ref halves
        for h in range(2):      # out rows 64h..64h+63, src tile (x|r)[h]
            cbase = b * 128 + h * 64
            for t in range(9):
                mat = dwp[b * 18 + h * 9 + t]
                m = np.arange(64)
                mat[2 * m, m] = dww[cbase + m, 0, t]
                mat[2 * m + 1, m] = dww[cbase + m, 1, t]

    rep = np.zeros((2, KP, 128), np.float32)
    for gb in range(2):
        for k in range(KP):
            g = k // 9
            if gb * 4 <= g < gb * 4 + 4:
                m0 = (g - gb * 4) * 32
                rep[gb, k, m0 : m0 + 32] = 1.0

    b16 = lambda a: np.ascontiguousarray(np.asarray(a, np.float32)).astype(ml_dtypes.bfloat16)
    f32 = lambda a: np.ascontiguousarray(np.asarray(a, np.float32))
    return {
        "lv": b16(lv), "lo": b16(lo), "lom": b16(lom),
        "dwp": b16(dwp),
        "bv": f32(value_b).reshape(C, 1), "bo": f32(out_b).reshape(C, 1),
        "bf": f32(bfeat).reshape(3 * KP, 1),
        "rep": b16(rep), "idm": b16(np.eye(128, dtype=np.float32)),
        "kc": np.tile(np.array([2.0, 1.0, -1.0, -2.0], np.float32), (KP, 1)),
    }


_CACHE = {}


def kernel(**inputs):
    dbg = bool(inputs.pop("_dbg", False))
    trace = bool(inputs.pop("_trace", False))
    x = np.ascontiguousarray(np.asarray(inputs["input"], np.float32))
    r = np.ascontiguousarray(np.asarray(inputs["ref"], np.float32))
    wts = _prep_weights(inputs)

    key = ("nc", dbg)
    if key not in _CACHE:
        _CACHE[key] = _build_nc(dbg=dbg)
    nc = _CACHE[key]

    in_maps = []
    for c in range(NCORES):
        m = dict(wts)
        m["x"] = np.ascontiguousarray(x[c * S : (c + 1) * S])
        m["r"] = np.ascontiguousarray(r[c * S : (c + 1) * S])
        in_maps.append(m)

    res = run_bass_kernel_spmd(nc, in_maps, core_ids=list(range(NCORES)), trace=trace)
    kernel.last_results = res
    kernel.last_exec_ns = res.exec_time_ns
    y = np.concatenate([res.results[c]["y"] for c in range(NCORES)], axis=0)
    return y.reshape(N, C, H, W)


# revision 25
# speedup vs baseline: 1.0228x; 1.0156x over previous
"""DCNv4 Bass kernel for Trainium2, 8 NeuronCores, data-parallel over batch.

Per core (2 samples each), channels-on-partition / pixels-on-free layout:
  1. PE: value projection xv = value_w @ x  (into a zero-halo padded 60x60 grid)
  2. PE: grouped depthwise 3x3 conv as 9-tap paired matmuls (lhsT has the two
     per-channel weights at rows 2m/2m+1), col-tiled 2x64 so both output halves
     run concurrently; accumulated in PSUM f32.
  3. PE: offset/mask projection; om_w columns host-reordered to [ry | rx | mask]
     with grid offsets gy/gx and dw bias folded into the projection bias, so
     ry = gy + dy directly.  Features stored bf16.
  4. Tents t_k(u) = relu(1 - |u - k|), k in {-2,-1,1,2}; t_0 via partition of
     unity.  Computed PER PIXEL-CHUNK inside the sampling loop so they overlap
     with sampling of the previous chunk.  x-tents stored NEGATED
     (min(|u|-1, 0)) fully on DVE; the mask-fold uses -msk so the per-shift
     product (mty*txs) has the correct sign.
  5. PE: 0/1 matmul per shift = reduce-over-9-points + replicate-over-32-channels,
     giving composite weight A_s[(g,c), pix] in PSUM.
  6. DVE: A_s * xv_shifted products; PE: identity-matmul PSUM accumulation over
     the 25 composite shifts.  FSET shifts read A_s straight from PSUM (1x)
     skipping the ACT copy; GSET shifts compute their tent product on the
     otherwise-idle GPSIMD.  idm accumulation is emitted one shift late so the
     in-order PE never waits on the freshly produced vprod.
  7. PE: output projection, chunked with immediate DMA out.
"""

import numpy as np
import ml_dtypes

import concourse.bacc as bacc
import concourse.mybir as mybir
import concourse.tile as tile
from concourse.bass_utils import run_bass_kernel_spmd

F32 = mybir.dt.float32
BF16 = mybir.dt.bfloat16
AF = mybir.ActivationFunctionType
ALU = mybir.AluOpType

C, G, Cg = 256, 8, 32
N, H, W = 16, 56, 56
HW = H * W
NCORES = 8
S = N // NCORES
PAD = 2
Hp = Wp = H + 2 * PAD          # 60
KP = 72                        # G*9 point-features
NCH, NROWS = 448, 8            # projection N-chunk: 8 rows of 56
FCH, FROWS = 784, 14           # sampling-stage chunk: 14 rows of 56 (PSUM limit)
SHIFTS = [(sy, sx) for sy in range(-2, 3) for sx in range(-2, 3)]
FSET = {2, 7, 12, 17, 22}      # shifts whose vprod reads A from PSUM directly
GSET = {0, 5, 10, 15, 20}      # shifts whose tent product runs on GPSIMD


def _build_nc(dbg=False):
    nc = bacc.Bacc("TRN2", num_devices=NCORES)
    t_x = nc.dram_tensor("x", (S, C, H, W), F32, kind="ExternalInput")
    t_r = nc.dram_tensor("r", (S, C, H, W), F32, kind="ExternalInput")
    t_lv = nc.dram_tensor("lv", (C, C), BF16, kind="ExternalInput")
    t_lo = nc.dram_tensor("lo", (C, C), BF16, kind="ExternalInput")
    t_lom = nc.dram_tensor("lom", (C, 3 * KP), BF16, kind="ExternalInput")
    t_dwp = nc.dram_tensor("dwp", (36, 128, 64), BF16, kind="ExternalInput")
    t_bv = nc.dram_tensor("bv", (C, 1), F32, kind="ExternalInput")
    t_bo = nc.dram_tensor("bo", (C, 1), F32, kind="ExternalInput")
    t_bf = nc.dram_tensor("bf", (3 * KP, 1), F32, kind="ExternalInput")
    t_rep = nc.dram_tensor("rep", (2, KP, 128), BF16, kind="ExternalInput")
    t_kc = nc.dram_tensor("kc", (KP, 4), F32, kind="ExternalInput")
    t_id = nc.dram_tensor("idm", (128, 128), BF16, kind="ExternalInput")
    t_y = nc.dram_tensor("y", (S, C, H, W), F32, kind="ExternalOutput")
    dbg_t = {}
    if dbg:
        dbg_t["xv"] = nc.dram_tensor("dbg_xv", (S, C, Hp * Wp), F32, kind="ExternalOutput")
        dbg_t["dwf"] = nc.dram_tensor("dbg_dwf", (S, C, HW), F32, kind="ExternalOutput")
        dbg_t["feat"] = nc.dram_tensor("dbg_feat", (S, 3 * KP, HW), F32, kind="ExternalOutput")
        dbg_t["acc"] = nc.dram_tensor("dbg_acc", (S, C, HW), F32, kind="ExternalOutput")

    with tile.TileContext(nc) as tc:
        from contextlib import ExitStack
        ctx = ExitStack()
        wp = ctx.enter_context(tc.tile_pool(name="wts", bufs=1))
        lv = wp.tile([128, 2, C], BF16)
        nc.sync.dma_start(lv[:], t_lv[:].rearrange("(kc k) m -> k kc m", k=128))
        lo = wp.tile([128, 2, C], BF16)
        nc.sync.dma_start(lo[:], t_lo[:].rearrange("(kc k) m -> k kc m", k=128))
        lom = wp.tile([128, 2, 3 * KP], BF16)
        nc.sync.dma_start(lom[:], t_lom[:].rearrange("(kc k) m -> k kc m", k=128))
        dwp = wp.tile([128, 36, 64], BF16)
        nc.sync.dma_start(dwp[:], t_dwp[:].rearrange("n k m -> k n m"))
        bv = wp.tile([128, 2], F32)
        nc.sync.dma_start(bv[:], t_bv[:].rearrange("(mc k) o -> k (mc o)", k=128))
        bo = wp.tile([128, 2], F32)
        nc.sync.dma_start(bo[:], t_bo[:].rearrange("(mc k) o -> k (mc o)", k=128))
        bf = wp.tile([KP, 3], F32)
        nc.sync.dma_start(bf[:], t_bf[:].rearrange("(f k) o -> k (f o)", k=KP))
        rep = wp.tile([KP, 2, 128], BF16)
        nc.sync.dma_start(rep[:], t_rep[:].rearrange("g k m -> k g m"))
        idm = wp.tile([128, 128], BF16)
        nc.sync.dma_start(idm[:], t_id[:])
        kc = wp.tile([KP, 4], F32)
        nc.sync.dma_start(kc[:], t_kc[:])

        ap = ctx.enter_context(tc.tile_pool(name="acts", bufs=1))
        sp = ctx.enter_context(tc.tile_pool(name="small", bufs=3))
        sp4 = ctx.enter_context(tc.tile_pool(name="small4", bufs=4))
        pp = ctx.enter_context(tc.tile_pool(name="ps", bufs=2, space="PSUM"))
        pacc = ctx.enter_context(tc.tile_pool(name="pacc", bufs=1, space="PSUM"))

        def halo_zero(t):
            nc.vector.memset(t[:, 0:PAD, :], 0.0)
            nc.vector.memset(t[:, Hp - PAD : Hp, :], 0.0)
            nc.vector.memset(t[:, PAD : Hp - PAD, 0:PAD], 0.0)
            nc.vector.memset(t[:, PAD : Hp - PAD, Wp - PAD : Wp], 0.0)

        for s in range(S):
            x0 = ap.tile([128, Hp, Wp], BF16, tag="x0")
            x1 = ap.tile([128, Hp, Wp], BF16, tag="x1")
            r0 = ap.tile([128, Hp, Wp], BF16, tag="r0")
            r1 = ap.tile([128, Hp, Wp], BF16, tag="r1")
            for t in (x0, x1, r0, r1):
                halo_zero(t)
            src_x = t_x[s].rearrange("(par c) h w -> par c h w", par=2)
            src_r = t_r[s].rearrange("(par c) h w -> par c h w", par=2)
            nc.gpsimd.dma_start(x0[:, PAD : PAD + H, PAD : PAD + W], src_x[0])
            nc.gpsimd.dma_start(x1[:, PAD : PAD + H, PAD : PAD + W], src_x[1])
            nc.gpsimd.dma_start(r0[:, PAD : PAD + H, PAD : PAD + W], src_r[0])
            nc.gpsimd.dma_start(r1[:, PAD : PAD + H, PAD : PAD + W], src_r[1])

            def win(t, dy=0, dx=0, r0=0, nr=H):
                return t[:, PAD + dy + r0 : PAD + dy + r0 + nr, PAD + dx : PAD + dx + W]

            # ---- 1. value projection ----
            xvc = ap.tile([128, 2, Hp, Wp], BF16, tag="xv", name="xv")
            xv = [xvc[:, m] for m in range(2)]
            for m in range(2):
                halo_zero(xv[m])
            for m in range(2):
                for q in range(7):
                    ps = pp.tile([128, 1024], F32, tag="mm")
                    psv = ps[:, :NCH]
                    nc.tensor.matmul(psv, lv[:, 0, 128 * m : 128 * m + 128],
                                     win(x0, r0=NROWS * q, nr=NROWS), start=True, stop=False)
                    nc.tensor.matmul(psv, lv[:, 1, 128 * m : 128 * m + 128],
                                     win(x1, r0=NROWS * q, nr=NROWS), start=False, stop=True)
                    nc.scalar.activation(win(xv[m], r0=NROWS * q, nr=NROWS),
                                         psv.rearrange("p (h w) -> p h w", w=W),
                                         AF.Identity, bias=bv[:, m : m + 1], scale=1.0)

            # ---- 2. depthwise conv on PE: 9-tap paired matmuls, 2x64 col-tiled ----
            dwf = [ap.tile([128, HW], BF16, tag=f"dwf{b}", name=f"dwf{b}") for b in range(2)]
            for b, (ta, tb) in enumerate(((x0, x1), (r0, r1))):
                for q in range(7):
                    ps = pp.tile([128, 1024], F32, tag="mm")
                    psd = ps[:, :NCH]
                    for t in range(9):
                        i, j = t // 3, t % 3
                        for h, src in enumerate((ta, tb)):
                            nc.tensor.matmul(
                                psd[64 * h : 64 * h + 64, :],
                                dwp[:, b * 18 + h * 9 + t, :],
                                win(src, dy=i - 1, dx=j - 1, r0=NROWS * q, nr=NROWS),
                                start=(t == 0), stop=(t == 8),
                                tile_position=(0, 64 * h), skip_group_check=True)
                    nc.scalar.activation(dwf[b][:, NCH * q : NCH * q + NCH], psd,
                                         AF.Copy, scale=1.0)

            # xv shifted left by one column: makes every sampling-mul window
            # 4B-aligned (odd dx reads xvd at dx-1), keeping DVE in 2x mode
            xvdc = ap.tile([128, 2, Hp, Wp], BF16, tag="xvd", name="xvd")
            xvd = [xvdc[:, m] for m in range(2)]
            nc.vector.tensor_copy(xvdc[:, :, :, 0 : Wp - 1], xvc[:, :, :, 1:Wp])

            # ---- 3. offset/mask projection (features in bf16) ----
            feat = [ap.tile([KP, HW], BF16, tag=f"feat{f}", name=f"feat{f}") for f in range(3)]
            for f in range(3):
                for q in range(7):
                    ps = pp.tile([128, 1024], F32, tag="mm")
                    psf = ps[:KP, :NCH]
                    nc.tensor.matmul(psf, lom[:, 0, KP * f : KP * f + KP],
                                     dwf[0][:, NCH * q : NCH * q + NCH], start=True, stop=False)
                    nc.tensor.matmul(psf, lom[:, 1, KP * f : KP * f + KP],
                                     dwf[1][:, NCH * q : NCH * q + NCH], start=False, stop=True)
                    nc.scalar.activation(feat[f][:, NCH * q : NCH * q + NCH], psf,
                                         AF.Identity, bias=bf[:, f : f + 1], scale=1.0)
            ry, rx, msk = feat

            # ---- 4. tents (emitted per pixel-chunk inside the sampling loop) ----
            nmsk16 = ap.tile([KP, HW], BF16, tag="msk16")
            nc.vector.tensor_scalar(nmsk16[:], msk[:], -1.0, None, op0=ALU.mult)
            mty = {
                -2: ap.tile([KP, HW], BF16, tag="dwf0", name="mty-2"),
                -1: ap.tile([KP, HW], BF16, tag="dwf1", name="mty-1"),
                0: ap.tile([KP, HW], BF16, tag="mty0", name="mty0"),
                1: ap.tile([KP, HW], BF16, tag="mty1", name="mty1"),
                2: ap.tile([KP, HW], BF16, tag="mty2", name="mty2"),
            }
            txs = {
                -2: ap.tile([KP, HW], BF16, tag="txs-2", name="txs-2"),
                -1: ap.tile([KP, HW], BF16, tag="txs-1", name="txs-1"),
                0: ap.tile([KP, HW], BF16, tag="txs0", name="txs0"),
                1: ap.tile([KP, HW], BF16, tag="txs1", name="txs1"),
                2: ap.tile([KP, HW], BF16, tag="txs2", name="txs2"),
            }

            def emit_tents(col, ncol):
                rxc = rx[:, col : col + ncol]
                ryc = ry[:, col : col + ncol]
                # x-tents on DVE: -t_k = min(max(u-1, -u-1), 0), u = rx-k
                for k in (-2, -1, 1, 2):
                    th = sp.tile([KP, ncol], BF16, tag="tc16")
                    dst = txs[k][:, col : col + ncol]
                    nc.vector.tensor_scalar(th[:], rxc, float(-k - 1), None, op0=ALU.add)
                    nc.vector.tensor_scalar(dst, rxc, -1.0, float(k - 1), op0=ALU.mult, op1=ALU.add)
                    nc.vector.tensor_tensor(dst, dst, th[:], op=ALU.max)
                    nc.vector.tensor_scalar(dst, dst, 0.0, None, op0=ALU.min)
                # y-tents on ACT, negated-mask fold on DVE
                for kj, k in enumerate((-2, -1, 1, 2)):
                    kb = kc[:, kj : kj + 1]
                    tmpc = sp.tile([KP, ncol], BF16, tag="tmpc")
                    t16 = sp.tile([KP, ncol], BF16, tag="tc16")
                    nc.scalar.activation(tmpc[:], ryc, AF.Abs, bias=kb, scale=1.0)
                    nc.scalar.activation(t16[:], tmpc[:], AF.Relu, bias=1.0, scale=-1.0)
                    nc.vector.tensor_mul(mty[k][:, col : col + ncol],
                                         nmsk16[:, col : col + ncol], t16[:])
                # partition of unity
                s01 = sp.tile([KP, ncol], BF16, tag="s01c")
                nc.vector.tensor_add(s01[:], mty[-2][:, col : col + ncol], mty[-1][:, col : col + ncol])
                nc.vector.tensor_add(s01[:], s01[:], mty[1][:, col : col + ncol])
                nc.vector.tensor_add(s01[:], s01[:], mty[2][:, col : col + ncol])
                nc.vector.tensor_tensor(mty[0][:, col : col + ncol],
                                        nmsk16[:, col : col + ncol], s01[:], op=ALU.subtract)
                s01b = sp.tile([KP, ncol], BF16, tag="s01c")
                nc.vector.tensor_add(s01b[:], txs[-2][:, col : col + ncol], txs[-1][:, col : col + ncol])
                nc.vector.tensor_add(s01b[:], s01b[:], txs[1][:, col : col + ncol])
                nc.vector.tensor_add(s01b[:], s01b[:], txs[2][:, col : col + ncol])
                nc.scalar.activation(txs[0][:, col : col + ncol], s01b[:],
                                     AF.Copy, bias=-1.0, scale=-1.0)

            # ---- 5+6. deformable sampling ----
            acc16 = [ap.tile([128, HW], BF16, tag=f"acc16_{gb}", name=f"acc16_{gb}") for gb in range(2)]
            for hc in range(4):
                col = FCH * hc
                emit_tents(col, FCH)
                accp = [[pacc.tile([128, 392], F32, tag=f"acc{gb}{hh}", name=f"accp{gb}{hh}")
                         for hh in range(2)] for gb in range(2)]

                def emit_prod(si):
                    sy, sx = SHIFTS[si]
                    prod = sp.tile([KP, FCH], BF16, tag="prod")
                    src0 = mty[sy][:, col : col + FCH]
                    src1 = txs[sx][:, col : col + FCH]
                    if si in GSET:
                        nc.gpsimd.tensor_tensor(prod[:], src0, src1, op=ALU.mult)
                    else:
                        nc.vector.tensor_mul(prod[:], src0, src1)
                    return prod

                def emit_idm(psi, pv, last=False):
                    for gb in range(2):
                        for hh in range(2):
                            nc.tensor.matmul(accp[gb][hh][:], idm[:],
                                             pv[:, gb, 392 * hh : 392 * hh + 392],
                                             start=(psi == 0), stop=last)

                pend = []   # [(si, vpcat)] awaiting idm accumulation (depth 2)
                prods = {0: emit_prod(0)}
                for si, (sy, sx) in enumerate(SHIFTS):
                    if si + 1 < len(SHIFTS):
                        prods[si + 1] = emit_prod(si + 1)
                    prod = prods.pop(si)
                    vpcat = sp4.tile([128, 2, FCH], BF16, tag="vp")
                    # windows over BOTH gb halves at once: [p, 2, FROWS, W]
                    r0_ = PAD + sy + FROWS * hc
                    c0_ = PAD + sx if sx % 2 == 0 else PAD + sx - 1
                    xsv = xvc if sx % 2 == 0 else xvdc
                    xsrc2 = xsv[:, :, r0_ : r0_ + FROWS, c0_ : c0_ + W]
                    apss = []
                    for gb in range(2):
                        aps = pp.tile([128, 1024], F32, tag="mm")
                        for hh in range(2):
                            nc.tensor.matmul(aps[:, 512 * hh : 512 * hh + 392], rep[:, gb, :],
                                             prod[:, 392 * hh : 392 * hh + 392],
                                             start=True, stop=True)
                        apss.append(aps)
                    if si in FSET:
                        # direct-PSUM read on DVE (1x), skipping the ACT copy
                        for gb in range(2):
                            av = apss[gb][:].rearrange("p (b f) -> p b f", f=512)[:, :, :392] \
                                .rearrange("p b (r w) -> p b r w", w=W)
                            nc.vector.tensor_tensor(
                                vpcat[:, gb].rearrange("p (b r w) -> p b r w", r=FROWS // 2, w=W),
                                av,
                                xsrc2[:, gb].rearrange("p (b r) w -> p b r w", b=2),
                                op=ALU.mult)
                    else:
                        a16c = sp.tile([128, 2, FCH], BF16, tag="a16")
                        for gb in range(2):
                            nc.scalar.activation(
                                a16c[:, gb].rearrange("p (b f) -> p b f", f=392),
                                apss[gb][:].rearrange("p (b f) -> p b f", f=512)[:, :, :392],
                                AF.Copy, scale=1.0)
                        # ONE 2x-mode multiply over both gb halves (FD=1568)
                        nc.vector.tensor_tensor(
                            vpcat[:].rearrange("p g (r w) -> p g r w", w=W),
                            a16c[:].rearrange("p g (r w) -> p g r w", w=W),
                            xsrc2, op=ALU.mult)
                    pend.append((si, vpcat))
                    # accumulate two shifts late so the in-order PE never waits
                    if len(pend) > 2:
                        emit_idm(*pend.pop(0))
                for psi, pvps in pend:
                    emit_idm(psi, pvps, last=(psi == len(SHIFTS) - 1))
                for gb in range(2):
                    for hh in range(2):
                        nc.scalar.activation(acc16[gb][:, col + 392 * hh : col + 392 * hh + 392],
                                             accp[gb][hh][:], AF.Copy, scale=1.0)

            # ---- 7. output projection, chunked with immediate DMA ----
            yq = t_y[s].rearrange("(mc k) h w -> mc k h w", mc=2)
            for m in range(2):
                for q in range(7):
                    ps = pp.tile([128, 1024], F32, tag="mm")
                    psv = ps[:, :NCH]
                    nc.tensor.matmul(psv, lo[:, 0, 128 * m : 128 * m + 128],
                                     acc16[0][:, NCH * q : NCH * q + NCH], start=True, stop=False)
                    nc.tensor.matmul(psv, lo[:, 1, 128 * m : 128 * m + 128],
                                     acc16[1][:, NCH * q : NCH * q + NCH], start=False, stop=True)
                    ytc = sp.tile([128, NCH], F32, tag="ytc")
                    nc.scalar.activation(ytc[:], psv, AF.Identity, bias=bo[:, m : m + 1], scale=1.0)
                    nc.sync.dma_start(yq[m, :, NROWS * q : NROWS * q + NROWS, :],
                                      ytc[:].rearrange("p (h w) -> p h w", w=W))

            if dbg:
                for m in range(2):
                    nc.gpsimd.dma_start(dbg_t["xv"][s, 128 * m : 128 * m + 128],
                                      xv[m][:].rearrange("p h w -> p (h w)"))
                    nc.gpsimd.dma_start(dbg_t["dwf"][s, 128 * m : 128 * m + 128], dwf[m][:])
                    nc.gpsimd.dma_start(dbg_t["acc"][s, 128 * m : 128 * m + 128], acc16[m][:])
                for f in range(3):
                    nc.sync.dma_start(dbg_t["feat"][s, KP * f : KP * f + KP], feat[f][:])
        ctx.close()
    nc.compile()
    return nc


def _prep_weights(inputs):
    value_w = np.asarray(inputs["value_w"], np.float32)
    out_w = np.asarray(inputs["out_w"], np.float32)
    om_w = np.asarray(inputs["om_w"], np.float32)
    dw_w = np.asarray(inputs["dw_w"], np.float32)
    value_b = np.asarray(inputs["value_b"], np.float32)
    out_b = np.asarray(inputs["out_b"], np.float32)
    om_b = np.asarray(inputs["om_b"], np.float32)
    dw_b = np.asarray(inputs["dw_b"], np.float32)

    lv = value_w.T
    lo = out_w.T

    gidx = np.arange(G)[:, None]
    p = np.arange(9)[None, :]
    col_ry = (gidx * 27 + 2 * p + 1).reshape(-1)
    col_rx = (gidx * 27 + 2 * p).reshape(-1)
    col_mk = (gidx * 27 + 18 + p).reshape(-1)
    cols = np.concatenate([col_ry, col_rx, col_mk])
    lom = om_w.T[:, cols]
    gy = np.tile((np.arange(9) // 3 - 1).astype(np.float32), G)
    gx = np.tile((np.arange(9) % 3 - 1).astype(np.float32), G)
    bfeat = np.concatenate([om_b[col_ry] + gy, om_b[col_rx] + gx, om_b[col_mk]])
    bfeat = bfeat + (om_w @ dw_b)[cols]

    # paired depthwise weights: dwp[b*18 + h*9 + t] is [128 src-part, 64 out]
    # dw output channel cbase+m uses src channels 2m, 2m+1 of its half-tile
    dww = dw_w.reshape(C, 2, 9)
    dwp = np.zeros((36, 128, 64), np.float32)
    for b in range(2):          # dwf0 <- input halves, dwf1 <- ref halves
        for h in range(2):      # out rows 64h..64h+63, src tile (x|r)[h]
            cbase = b * 128 + h * 64
            for t in range(9):
                mat = dwp[b * 18 + h * 9 + t]
                m = np.arange(64)
                mat[2 * m, m] = dww[cbase + m, 0, t]
                mat[2 * m + 1, m] = dww[cbase + m, 1, t]

    rep = np.zeros((2, KP, 128), np.float32)
    for gb in range(2):
        for k in range(KP):
            g = k // 9
            if gb * 4 <= g < gb * 4 + 4:
                m0 = (g - gb * 4) * 32
                rep[gb, k, m0 : m0 + 32] = 1.0

    b16 = lambda a: np.ascontiguousarray(np.asarray(a, np.float32)).astype(ml_dtypes.bfloat16)
    f32 = lambda a: np.ascontiguousarray(np.asarray(a, np.float32))
    return {
        "lv": b16(lv), "lo": b16(lo), "lom": b16(lom),
        "dwp": b16(dwp),
        "bv": f32(value_b).reshape(C, 1), "bo": f32(out_b).reshape(C, 1),
        "bf": f32(bfeat).reshape(3 * KP, 1),
        "rep": b16(rep), "idm": b16(np.eye(128, dtype=np.float32)),
        "kc": np.tile(np.array([2.0, 1.0, -1.0, -2.0], np.float32), (KP, 1)),
    }


_CACHE = {}


def kernel(**inputs):
    dbg = bool(inputs.pop("_dbg", False))
    trace = bool(inputs.pop("_trace", False))
    x = np.ascontiguousarray(np.asarray(inputs["input"], np.float32))
    r = np.ascontiguousarray(np.asarray(inputs["ref"], np.float32))
    wts = _prep_weights(inputs)

    key = ("nc", dbg)
    if key not in _CACHE:
        _CACHE[key] = _build_nc(dbg=dbg)
    nc = _CACHE[key]

    in_maps = []
    for c in range(NCORES):
        m = dict(wts)
        m["x"] = np.ascontiguousarray(x[c * S : (c + 1) * S])
        m["r"] = np.ascontiguousarray(r[c * S : (c + 1) * S])
        in_maps.append(m)

    res = run_bass_kernel_spmd(nc, in_maps, core_ids=list(range(NCORES)), trace=trace)
    kernel.last_results = res
    kernel.last_exec_ns = res.exec_time_ns
    y = np.concatenate([res.results[c]["y"] for c in range(NCORES)], axis=0)
    return y.reshape(N, C, H, W)
